# revision 33
# baseline (speedup 1.0000x reference)
"""Self-contained Trainium2 Bass kernel for batched multi-head attention
with interleaved RoPE and a block-causal mask (block size 8).

Shapes (hardcoded): x [8, 1024, 1024] f32, weights [1024, 1024] f32,
freqs_cos/sin [1024, 32] f32 -> out [8, 1024, 1024] f32.

Sharding: data-parallel over batch, one batch element per NeuronCore (8 cores).

Device algorithm (per core, matmuls in bf16):
  - host pre-transposes x -> XT [D, S] and de-interleaves the RoPE pairing by
    permuting wq/wk columns so each head's 64 dims are [32 real | 32 imag].
  - QT = Wq^T XT, KT = Wk^T XT  ([D, S] layouts, head-major rows)
  - RoPE fused with the PSUM evacuation: tc = pq*cosf and ts = pq*sinf2
    (sinf2 is the 32-row-block-swapped sin table with signs folded), the
    32-row block swap is an SBUF->SBUF DMA on ts, and qt = tc + swap(ts) on
    the DVE (emitted one filler-unit later so the DVE never head-of-line
    stalls on the swap DMA).
  - V = XT^T Wv in natural [S, D] layout with a ones-column per head
    (V' [S, 65]) so the PV matmul also produces the softmax denominator.
  - scores transposed ST[k, q] per head, k-tiles packed into 5 super-steps
    {0},{1,7},{2,6},{3,5},{4} whose staircase widths sum to exactly <=1024,
    so each (head, step) is ONE [128,1024] PSUM tile and ONE exp -- 5 ACT
    ops per head instead of 8. Both heads' score matmuls are interleaved
    piece-by-piece (row groups 0-1 / 2-3).
  - exp on ACT with the 1/8 scale folded in; block-diagonal mask applied
    multiplicatively on the DVE per k-tile segment.
  - outT[h] = V'^T @ PT accumulated per 512-query bank in PSUM ([65, 512]);
    PV matmuls trail the exp by 2 steps (jb0) / 3 steps (jb1) so the PE
    never embeds exp-latency waits.
  - normalization per head-bank: ACT copy of the PSUM ones-row (ACT is
    exp-idle in steps 5..7), DVE reciprocal_approx_fast, GPSIMD
    partition-broadcast (the only gp op type -- a second one causes
    LIBRARY_RELOAD thrash), one DVE multiply PSUM->SBUF.
  - final = outT^T @ Wo in bf16, upcast to f32 on the host.

PE density: ~10us of warm-up matmuls on a zeroed tile run during the input
DMA lead-in so the HAM clock gate is at 8/8 before the V projection starts.
The attention loops are emission-interleaved with the remaining Q/K
projection groups (QK(t+1..) fire inside att(t) at steps 1/3) so the tensor
engine always has ready work while ACT runs the exp chain. PSUM budget: one
shared [128,1024] ring (bufs=2, 4 banks) for all projections + attention
scores, and 4 single-bank [65,512] slots for the two in-flight heads' PV
accumulators.
"""

import os
import sys
import types

import numpy as np

B, S, D, H, HD, BS = 8, 1024, 1024, 16, 64, 8
P = 128
NT = D // P  # 8 partition tiles
NCORES = 8

LAST_RESULT = None  # BassKernelResults of the most recent run (for test harness)


def _install_axon_hooks():
    """Provide antenv.axon_hooks (NTFF profiling hook) when the image lacks it."""
    if "antenv.axon_hooks" in sys.modules:
        return
    try:
        import antenv
        from trn_agent_boot.trn_boot import _ntff_profile_via_ctypes

        mod = types.ModuleType("antenv.axon_hooks")
        hook = _ntff_profile_via_ctypes("/opt/axon/libaxon_pjrt.so")
        mod.get_axon_ntff_profile_hook = lambda: hook
        mod.set_axon_ntff_profile_hook = lambda h: None
        sys.modules["antenv.axon_hooks"] = mod
        antenv.axon_hooks = mod
    except Exception:
        mod = types.ModuleType("antenv.axon_hooks")
        mod.get_axon_ntff_profile_hook = lambda: None
        mod.set_axon_ntff_profile_hook = lambda h: None
        sys.modules["antenv.axon_hooks"] = mod


_NC_CACHE = {}


def _build_nc():
    """Build and compile the Bass graph (one SPMD program for all 8 cores)."""
    if "nc" in _NC_CACHE:
        return _NC_CACHE["nc"]

    import concourse.mybir as mybir
    import concourse.tile as tile
    from concourse import bacc

    BF = mybir.dt.bfloat16
    F32 = mybir.dt.float32
    MUL = mybir.AluOpType.mult
    ADD = mybir.AluOpType.add
    EXP = mybir.ActivationFunctionType.Exp

    nc = bacc.Bacc("TRN2", target_bir_lowering=False, debug=False)

    xt_d = nc.dram_tensor("xt", [D, S], BF, kind="ExternalInput")
    wq_d = nc.dram_tensor("wq", [D, D], BF, kind="ExternalInput")
    wk_d = nc.dram_tensor("wk", [D, D], BF, kind="ExternalInput")
    wv_d = nc.dram_tensor("wv", [D, D], BF, kind="ExternalInput")
    wo_d = nc.dram_tensor("wo", [D, D], BF, kind="ExternalInput")
    cos_d = nc.dram_tensor("cosf", [P, S], BF, kind="ExternalInput")
    sin2_d = nc.dram_tensor("sinf2", [P, S], BF, kind="ExternalInput")
    mask_d = nc.dram_tensor("mask", [P, P], BF, kind="ExternalInput")
    out_d = nc.dram_tensor("out", [S, D], BF, kind="ExternalOutput")

    HC = HD + 1  # 65: V columns per head incl. the ones column

    with tile.TileContext(nc) as tc:
        with (
            tc.tile_pool(name="big", bufs=1) as big,
            tc.tile_pool(name="mmp", bufs=2, space="PSUM") as mmp,
            tc.tile_pool(name="otp_pool", bufs=4, space="PSUM") as otp_pool,
            tc.tile_pool(name="work", bufs=2) as work,
            tc.tile_pool(name="ptp", bufs=3) as ptp,
        ):
            xt = [big.tile([P, S], BF, tag=f"xt{j}", name=f"xt{j}") for j in range(NT)]
            wqt = [big.tile([P, D], BF, tag=f"wq{j}", name=f"wq{j}") for j in range(NT)]
            wkt = [big.tile([P, D], BF, tag=f"wk{j}", name=f"wk{j}") for j in range(NT)]
            wvt = [big.tile([P, D], BF, tag=f"wv{j}", name=f"wv{j}") for j in range(NT)]
            wot = [big.tile([P, D], BF, tag=f"wo{j}", name=f"wo{j}") for j in range(NT)]
            qt = [big.tile([P, S], BF, tag=f"qt{t}", name=f"qt{t}") for t in range(NT)]
            kt = [big.tile([P, S], BF, tag=f"kt{t}", name=f"kt{t}") for t in range(NT)]
            vs = [big.tile([P, H * HC], BF, tag=f"vs{t}", name=f"vs{t}") for t in range(NT)]
            ot = [big.tile([P, S], BF, tag=f"ot{t}", name=f"ot{t}") for t in range(NT)]
            cosf = big.tile([P, S], BF, tag="cosf", name="cosf")
            sinf2 = big.tile([P, S], BF, tag="sinf2", name="sinf2")
            maskt = big.tile([P, P], BF, tag="mask", name="mask")

            # load order matters for the compute ramp: xt+wv feed the V
            # projection (needed before any attention), wq/wk next, wo last
            # column-half DMA split (full 128 partitions per transfer, so no
            # partial-partition bandwidth penalty): all first-halves land in
            # one queue round (~6us) and V0's m=0 matmuls -- which only read
            # xt cols 0:128 and wv cols 0:512 -- start a full round earlier
            for half in (0, 1):
                cs2 = slice(half * 512, (half + 1) * 512)
                for j in range(NT):
                    rs = slice(j * P, (j + 1) * P)
                    nc.sync.dma_start(xt[j][:, cs2], xt_d[rs, cs2])
                    nc.sync.dma_start(wvt[j][:, cs2], wv_d[rs, cs2])
            for j in range(NT):
                rs = slice(j * P, (j + 1) * P)
                nc.sync.dma_start(wqt[j][:], wq_d[rs, :])
                nc.sync.dma_start(wkt[j][:], wk_d[rs, :])
            nc.sync.dma_start(cosf[:], cos_d[:])
            nc.sync.dma_start(sinf2[:], sin2_d[:])
            nc.sync.dma_start(maskt[:], mask_d[:])
            for j in range(NT):
                rs = slice(j * P, (j + 1) * P)
                nc.sync.dma_start(wot[j][:], wo_d[rs, :])

            for t in range(NT):
                nc.vector.memset(
                    vs[t].rearrange("p (h c) -> p h c", c=HC)[:, :, HD : HD + 1], 1.0
                )

            # warm-up matmuls on a zeroed tile during the input-DMA lead-in:
            # ~10us of PE activity flips the HAM clock gate to 8/8 before the
            # V projection starts, so nothing runs at the 1.2 GHz cold clock
            warm = big.tile([P, 512], BF, tag="warm", name="warm")
            nc.vector.memset(warm[:], 0.0)
            wps = mmp.tile([P, 1024], F32, tag="mm", name="wps")
            for wi in range(10):
                nc.tensor.matmul(
                    wps[:, 0:512], warm[:, 0:P], warm[:], start=True, stop=True
                )
            nc.scalar.copy(warm[0:1, 0:1], wps[0:1, 0:1])

            # ---------------- projection emitters (one [128,1024] slot each)
            def v_proj(t):
                cs = slice(t * P, (t + 1) * P)
                pv = mmp.tile([P, 1024], F32, tag="mm", name="pv")
                for m in range(2):
                    sl = slice(m * 512, (m + 1) * 512)
                    for j in range(NT):
                        nc.tensor.matmul(
                            pv[:, sl], xt[j][:, cs], wvt[j][:, sl],
                            start=(j == 0), stop=(j == NT - 1),
                        )
                dst = vs[t].rearrange("p (h c) -> p h c", c=HC)[:, :, 0:HD]
                srcv = pv.rearrange("p (h c) -> p h c", c=HD)
                # ACT is idle during the projection prologue; keep DVE free
                nc.scalar.copy(dst, srcv)

            def qk_mms(t, which):
                """Yield the 16 projection matmuls for Q or K of tile t as
                thunk-batches of 4 (filler units for the attention loop)."""
                cs = slice(t * P, (t + 1) * P)
                w = wqt if which == "q" else wkt
                pq = mmp.tile([P, 1024], F32, tag="mm", name=f"p{which}{t}")
                for m in range(2):
                    sl = slice(m * 512, (m + 1) * 512)
                    for j in range(NT):
                        nc.tensor.matmul(
                            pq[:, sl], w[j][:, cs], xt[j][:, sl],
                            start=(j == 0), stop=(j == NT - 1),
                        )
                return pq

            def rope_muls(t, which, pq):
                """RoPE part 1: evacuate PSUM with the cos/sin multiplies and
                start the 32-row-block swap DMA. Returns state for rope_add."""
                tcos = work.tile([P, S], BF, tag="tcos", name="tcos")
                tsin = work.tile([P, S], BF, tag="tsin", name="tsin")
                trot = work.tile([P, S], BF, tag="trot", name="trot")
                nc.vector.tensor_tensor(tcos[:], pq[:], cosf[:], op=MUL)
                nc.vector.tensor_tensor(tsin[:], pq[:], sinf2[:], op=MUL)
                for b4 in range(4):
                    sblk = (b4 ^ 1) * 32
                    dblk = b4 * 32
                    nc.sync.dma_start(
                        trot[dblk : dblk + 32, :], tsin[sblk : sblk + 32, :]
                    )
                return (t, which, tcos, trot)

            def rope_add(st):
                """RoPE part 2 (emitted later so the DVE doesn't head-of-line
                stall on the swap DMA). GPSIMD must stay single-program
                (PartitionBroadcast only) -- a second gp op type causes
                LIBRARY_RELOAD thrash + misexec, so this add stays on DVE."""
                t, which, tcos, trot = st
                buf_t = qt[t] if which == "q" else kt[t]
                nc.vector.tensor_tensor(buf_t[:], tcos[:], trot[:], op=ADD)

            def rope(t, which, pq):
                rope_add(rope_muls(t, which, pq))

            # ---------------- attention for one tile pair (2 heads) ---------
            # Emission pipeline per step s:
            #   ST_A(s), ST_B(s)                      (PE, row-group packed)
            #   exp+mask A(s), B(s)                   (ACT, DVE)
            #   PV_A(s-1) jb0, PV_B(s-1) jb0          (PE)
            #   PV_A(s-2) jb1, PV_B(s-2) jb1          (PE, trails for norm)
            #   filler()                              (projection MMs)
            scale = 1.0 / 8.0

            # k-tile super-steps: widths pack to exactly <=1024 score columns
            # per step, so each (head, step) is ONE [128,1024] PSUM tile and
            # ONE exp -- 5 ACT ops per head instead of 8.
            STEPS = [[0], [1, 7], [2, 6], [3, 5], [4]]
            NS = len(STEPS)

            def attention(t, filler):
                heads = []
                for hh in (0, 1):
                    heads.append({
                        "base": hh * HD,
                        "h": 2 * t + hh,
                        "otp": {
                            0: otp_pool.tile([HC, 512], F32, tag="ot", name="otp0"),
                            1: otp_pool.tile([HC, 512], F32, tag="ot", name="otp1"),
                        },
                        "pt": {},   # i -> (tile, col_offset_of_i_in_tile)
                    })

                def st_exp(hd, s):
                    base = hd["base"]
                    widths = [S - i * P for i in STEPS[s]]
                    tot = sum(widths)
                    stp = mmp.tile([P, 1024], F32, tag="mm", name="stp")
                    pt = ptp.tile([P, tot], BF, tag=f"pt{s}", name=f"pt{s}")
                    col = 0
                    for i, w in zip(STEPS[s], widths):
                        off = i * P
                        o = 0
                        while o < w:
                            wd = min(w - o, 512 - (col + o) % 512)
                            if (col + o) % 512 == 0:
                                wd = min(w - o, 512)
                            nc.tensor.matmul(
                                stp[:, col + o : col + o + wd],
                                kt[t][base : base + HD, off : off + P],
                                qt[t][base : base + HD, off + o : off + o + wd],
                                start=True, stop=True,
                            )
                            o += wd
                        hd["pt"][i] = (pt, col)
                        col += w
                    nc.scalar.activation(pt[:, :tot], stp[:, :tot], EXP, scale=scale)
                    col = 0
                    for i, w in zip(STEPS[s], widths):
                        nc.vector.tensor_tensor(
                            pt[:, col : col + P], pt[:, col : col + P],
                            maskt[:], op=MUL,
                        )
                        col += w

                def pv(hd, i, jb):
                    lo = jb * 512
                    if i > 4 * jb + 3:
                        return
                    off = i * P
                    o = max(lo, off)
                    wd = lo + 512 - o
                    pt, col = hd["pt"][i]
                    nc.tensor.matmul(
                        hd["otp"][jb][:, o - lo : o - lo + wd],
                        vs[i][:, hd["h"] * HC : (hd["h"] + 1) * HC],
                        pt[:, col + o - off : col + o - off + wd],
                        start=(i == 0),
                        stop=(i == (3 if jb == 0 else 4)),
                    )

                def norm_single(hd, jb):
                    sl = slice(jb * 512, (jb + 1) * 512)
                    den = work.tile([1, 512], F32, tag="den", name="den")
                    nc.scalar.copy(den[:], hd["otp"][jb][HD : HD + 1, :])
                    rec = work.tile([1, 512], F32, tag="rec", name="rec")
                    nc.vector.reciprocal_approx_fast(rec[:], den[:])
                    bc = work.tile([HD, 512], F32, tag="bc", name="bc")
                    nc.gpsimd.partition_broadcast(bc[:], rec[:])
                    nc.vector.tensor_tensor(
                        ot[t][hd["base"] : hd["base"] + HD, sl],
                        hd["otp"][jb][0:HD, :], bc[:],
                        op=MUL,
                    )

                # PV trails the exp by 2 steps (jb0) / 3 steps (jb1) so
                # the PE never embeds exp-latency waits in the PV matmuls
                for s in range(NS + 3):
                    if s < NS:
                        for hd in heads:
                            st_exp(hd, s)
                    for hd in heads:
                        if 0 <= s - 2 < NS:
                            for i in STEPS[s - 2]:
                                pv(hd, i, 0)
                    for hd in heads:
                        if 0 <= s - 3 < NS:
                            for i in STEPS[s - 3]:
                                pv(hd, i, 1)
                    if s == 5:
                        for hd in heads:
                            norm_single(hd, 0)
                    filler(s)
                for hd in heads:
                    norm_single(hd, 1)

            # ---------------- emission schedule -----------------------------
            # All V tiles + Q/K of tiles 0..1 (with rope) up front so
            # attention(0) starts with everything ready. Q/K 2..7 become PE
            # filler inside the attention loops: QK(t+2) fires during att(t),
            # two tiles ahead of its consumer. Each filler unit first emits
            # the PREVIOUS unit's rope-add (so the DVE never head-of-line
            # stalls on the swap DMA), then the next 16 projection matmuls +
            # rope multiplies.
            # dovetail the QK0/QK1q groups into the V phase right where the
            # wq/wk DMAs land (~22us): V0..V2 cover the xt+wv arrival window,
            # then the QK groups run while V3..V7 follow -- attention(0)
            # starts ~10us earlier than a strictly serial prologue
            for t in range(3):
                v_proj(t)
            rope(0, "q", qk_mms(0, "q"))
            rope(0, "k", qk_mms(0, "k"))
            rope(1, "q", qk_mms(1, "q"))
            for t in range(3, NT):
                v_proj(t)

            pending_add = [None]

            def flush_add():
                if pending_add[0] is not None:
                    rope_add(pending_add[0])
                    pending_add[0] = None

            def make_unit(t2, which):
                def u():
                    flush_add()
                    pq = qk_mms(t2, which)
                    pending_add[0] = rope_muls(t2, which, pq)
                return u

            filler_units = [make_unit(1, "k")] + [
                make_unit(t2, w) for t2 in range(2, NT) for w in ("q", "k")
            ]
            fill_iter = iter(filler_units)

            def filler(s):
                if s in (1, 3):
                    u = next(fill_iter, None)
                    if u is not None:
                        u()
                    else:
                        flush_add()
                elif s == 5:
                    flush_add()

            for t in range(NT):
                attention(t, filler)

            # ---- output projection: final[s, :] = sum_i ot[i][:, s]^T wo[i]
            for st in range(NT):
                cs = slice(st * P, (st + 1) * P)
                fp = mmp.tile([P, 1024], F32, tag="mm", name="fp")
                for m in range(2):
                    sl = slice(m * 512, (m + 1) * 512)
                    for i in range(NT):
                        nc.tensor.matmul(
                            fp[:, sl], ot[i][:, cs], wot[i][:, sl],
                            start=(i == 0), stop=(i == NT - 1),
                        )
                osb = work.tile([P, 1024], BF, tag="osb", name="osb")
                # tail phase: ACT is idle, keep DVE free
                nc.scalar.copy(osb[:], fp[:])
                nc.sync.dma_start(out_d[cs, :], osb[:])

    nc.compile()
    _NC_CACHE["nc"] = nc
    return nc


def _host_prep(x, wq, wk, wv, wo, freqs_cos, freqs_sin):
    import ml_dtypes

    bf16 = ml_dtypes.bfloat16

    # de-interleave RoPE pairs: permuted col c of head h maps to original
    # column h*64 + (2r if r<32 else 2(r-32)+1)
    r = np.arange(HD)
    src_local = np.where(r < 32, 2 * r, 2 * (r - 32) + 1)
    perm = (np.arange(H)[:, None] * HD + src_local[None, :]).reshape(-1)

    wq_p = np.ascontiguousarray(wq[:, perm]).astype(bf16)
    wk_p = np.ascontiguousarray(wk[:, perm]).astype(bf16)
    wv_c = np.ascontiguousarray(wv).astype(bf16)
    wo_c = np.ascontiguousarray(wo).astype(bf16)

    cos_t = np.ascontiguousarray(freqs_cos.T).astype(np.float32)  # [32, S]
    sin_t = np.ascontiguousarray(freqs_sin.T).astype(np.float32)
    cosf = np.concatenate([cos_t, cos_t, cos_t, cos_t], 0).astype(bf16)  # [128,S]
    # sinf2 = 32-row-block swap of the sign-folded sin table
    # (sinf = [-s, s, -s, s]  ->  sinf2 = [s, -s, s, -s])
    sinf2 = np.concatenate([sin_t, -sin_t, sin_t, -sin_t], 0).astype(bf16)

    kq = np.arange(P)
    mask = ((kq[:, None] // BS) <= (kq[None, :] // BS)).astype(bf16)  # [128,128]

    in_maps = []
    for b in range(NCORES):
        xt = np.ascontiguousarray(x[b].T).astype(bf16)  # [D, S]
        in_maps.append(
            {
                "xt": xt,
                "wq": wq_p,
                "wk": wk_p,
                "wv": wv_c,
                "wo": wo_c,
                "cosf": cosf,
                "sinf2": sinf2,
                "mask": mask,
            }
        )
    return in_maps


def kernel(x, wq, wk, wv, wo, freqs_cos, freqs_sin):
    global LAST_RESULT
    x = np.asarray(x, dtype=np.float32)
    wq = np.asarray(wq, dtype=np.float32)
    wk = np.asarray(wk, dtype=np.float32)
    wv = np.asarray(wv, dtype=np.float32)
    wo = np.asarray(wo, dtype=np.float32)
    freqs_cos = np.asarray(freqs_cos, dtype=np.float32)
    freqs_sin = np.asarray(freqs_sin, dtype=np.float32)

    trace = bool(os.environ.get("BASS_TRACE"))
    if trace:
        _install_axon_hooks()
        import concourse.bass_utils as bass_utils

        bass_utils.upload_artifacts = lambda tmpdir: tmpdir  # no-egress sandbox

    from concourse.bass_utils import run_bass_kernel_spmd

    nc = _build_nc()
    in_maps = _host_prep(x, wq, wk, wv, wo, freqs_cos, freqs_sin)
    res = run_bass_kernel_spmd(
        nc, in_maps, core_ids=list(range(NCORES)), trace=trace
    )
    LAST_RESULT = res
    out = np.stack([res.results[b]["out"] for b in range(NCORES)], 0)
    return out.astype(np.float32)


# revision 34
# speedup vs baseline: 1.0387x; 1.0387x over previous
"""Self-contained Trainium2 Bass kernel for batched multi-head attention
with interleaved RoPE and a block-causal mask (block size 8).

Shapes (hardcoded): x [8, 1024, 1024] f32, weights [1024, 1024] f32,
freqs_cos/sin [1024, 32] f32 -> out [8, 1024, 1024] f32.

Sharding: data-parallel over batch, one batch element per NeuronCore (8 cores).

Device algorithm (per core, matmuls in bf16):
  - host pre-transposes x -> XT [D, S] and de-interleaves the RoPE pairing by
    permuting wq/wk columns so each head's 64 dims are [32 real | 32 imag].
  - QT = Wq^T XT, KT = Wk^T XT  ([D, S] layouts, head-major rows)
  - RoPE fused with the PSUM evacuation: tc = pq*cosf and ts = pq*sinf2
    (sinf2 is the 32-row-block-swapped sin table with signs folded), the
    32-row block swap is an SBUF->SBUF DMA on ts, and qt = tc + swap(ts) on
    the DVE (emitted one filler-unit later so the DVE never head-of-line
    stalls on the swap DMA).
  - V = XT^T Wv in natural [S, D] layout with a ones-column per head
    (V' [S, 65]) so the PV matmul also produces the softmax denominator.
  - scores transposed ST[k, q] per head, k-tiles packed into 5 super-steps
    {0},{1,7},{2,6},{3,5},{4} whose staircase widths sum to exactly <=1024,
    so each (head, step) is ONE [128,1024] PSUM tile and ONE exp -- 5 ACT
    ops per head instead of 8. Both heads' score matmuls are interleaved
    piece-by-piece (row groups 0-1 / 2-3).
  - exp on ACT with the 1/8 scale folded in; block-diagonal mask applied
    multiplicatively on the DVE per k-tile segment.
  - outT[h] = V'^T @ PT accumulated per 512-query bank in PSUM ([65, 512]);
    PV matmuls trail the exp by 2 steps (jb0) / 3 steps (jb1) so the PE
    never embeds exp-latency waits.
  - normalization per head-bank: ACT copy of the PSUM ones-row (ACT is
    exp-idle in steps 5..7), DVE reciprocal_approx_fast, GPSIMD
    partition-broadcast (the only gp op type -- a second one causes
    LIBRARY_RELOAD thrash), one DVE multiply PSUM->SBUF.
  - final = outT^T @ Wo in bf16, upcast to f32 on the host.

PE density: ~10us of warm-up matmuls on a zeroed tile run during the input
DMA lead-in so the HAM clock gate is at 8/8 before the V projection starts.
The attention loops are emission-interleaved with the remaining Q/K
projection groups (QK(t+1..) fire inside att(t) at steps 1/3) so the tensor
engine always has ready work while ACT runs the exp chain. PSUM budget: one
shared [128,1024] ring (bufs=2, 4 banks) for all projections + attention
scores, and 4 single-bank [65,512] slots for the two in-flight heads' PV
accumulators.
"""

import os
import sys
import types

import numpy as np

B, S, D, H, HD, BS = 8, 1024, 1024, 16, 64, 8
P = 128
NT = D // P  # 8 partition tiles
NCORES = 8

LAST_RESULT = None  # BassKernelResults of the most recent run (for test harness)


def _install_axon_hooks():
    """Provide antenv.axon_hooks (NTFF profiling hook) when the image lacks it."""
    if "antenv.axon_hooks" in sys.modules:
        return
    try:
        import antenv
        from trn_agent_boot.trn_boot import _ntff_profile_via_ctypes

        mod = types.ModuleType("antenv.axon_hooks")
        hook = _ntff_profile_via_ctypes("/opt/axon/libaxon_pjrt.so")
        mod.get_axon_ntff_profile_hook = lambda: hook
        mod.set_axon_ntff_profile_hook = lambda h: None
        sys.modules["antenv.axon_hooks"] = mod
        antenv.axon_hooks = mod
    except Exception:
        mod = types.ModuleType("antenv.axon_hooks")
        mod.get_axon_ntff_profile_hook = lambda: None
        mod.set_axon_ntff_profile_hook = lambda h: None
        sys.modules["antenv.axon_hooks"] = mod


_NC_CACHE = {}


def _build_nc():
    """Build and compile the Bass graph (one SPMD program for all 8 cores)."""
    if "nc" in _NC_CACHE:
        return _NC_CACHE["nc"]

    import concourse.mybir as mybir
    import concourse.tile as tile
    from concourse import bacc

    BF = mybir.dt.bfloat16
    F32 = mybir.dt.float32
    MUL = mybir.AluOpType.mult
    ADD = mybir.AluOpType.add
    EXP = mybir.ActivationFunctionType.Exp

    nc = bacc.Bacc("TRN2", target_bir_lowering=False, debug=False)

    xt_d = nc.dram_tensor("xt", [D, S], BF, kind="ExternalInput")
    wq_d = nc.dram_tensor("wq", [D, D], BF, kind="ExternalInput")
    wk_d = nc.dram_tensor("wk", [D, D], BF, kind="ExternalInput")
    wv_d = nc.dram_tensor("wv", [D, D], BF, kind="ExternalInput")
    wo_d = nc.dram_tensor("wo", [D, D], BF, kind="ExternalInput")
    cos_d = nc.dram_tensor("cosf", [P, S], BF, kind="ExternalInput")
    sin2_d = nc.dram_tensor("sinf2", [P, S], BF, kind="ExternalInput")
    mask_d = nc.dram_tensor("mask", [P, P], BF, kind="ExternalInput")
    out_d = nc.dram_tensor("out", [S, D], BF, kind="ExternalOutput")

    HC = HD + 1  # 65: V columns per head incl. the ones column

    with tile.TileContext(nc) as tc:
        with (
            tc.tile_pool(name="big", bufs=1) as big,
            tc.tile_pool(name="mmp", bufs=2, space="PSUM") as mmp,
            tc.tile_pool(name="otp_pool", bufs=4, space="PSUM") as otp_pool,
            tc.tile_pool(name="work", bufs=2) as work,
            tc.tile_pool(name="ptp", bufs=3) as ptp,
        ):
            xt = [big.tile([P, S], BF, tag=f"xt{j}", name=f"xt{j}") for j in range(NT)]
            wqt = [big.tile([P, D], BF, tag=f"wq{j}", name=f"wq{j}") for j in range(NT)]
            wkt = [big.tile([P, D], BF, tag=f"wk{j}", name=f"wk{j}") for j in range(NT)]
            wvt = [big.tile([P, D], BF, tag=f"wv{j}", name=f"wv{j}") for j in range(NT)]
            wot = [big.tile([P, D], BF, tag=f"wo{j}", name=f"wo{j}") for j in range(NT)]
            qt = [big.tile([P, S], BF, tag=f"qt{t}", name=f"qt{t}") for t in range(NT)]
            kt = [big.tile([P, S], BF, tag=f"kt{t}", name=f"kt{t}") for t in range(NT)]
            vs = [big.tile([P, H * HC], BF, tag=f"vs{t}", name=f"vs{t}") for t in range(NT)]
            ot = [big.tile([P, S], BF, tag=f"ot{t}", name=f"ot{t}") for t in range(NT)]
            cosf = big.tile([P, S], BF, tag="cosf", name="cosf")
            sinf2 = big.tile([P, S], BF, tag="sinf2", name="sinf2")
            maskt = big.tile([P, P], BF, tag="mask", name="mask")

            # load order matters for the compute ramp: xt+wv feed the V
            # projection (needed before any attention), wq/wk next, wo last
            for j in range(NT):
                rs = slice(j * P, (j + 1) * P)
                nc.sync.dma_start(xt[j][:], xt_d[rs, :])
                nc.sync.dma_start(wvt[j][:], wv_d[rs, :])
            for j in range(NT):
                rs = slice(j * P, (j + 1) * P)
                nc.sync.dma_start(wqt[j][:], wq_d[rs, :])
                nc.sync.dma_start(wkt[j][:], wk_d[rs, :])
            nc.sync.dma_start(cosf[:], cos_d[:])
            nc.sync.dma_start(sinf2[:], sin2_d[:])
            nc.sync.dma_start(maskt[:], mask_d[:])
            for j in range(NT):
                rs = slice(j * P, (j + 1) * P)
                nc.sync.dma_start(wot[j][:], wo_d[rs, :])

            for t in range(NT):
                nc.vector.memset(
                    vs[t].rearrange("p (h c) -> p h c", c=HC)[:, :, HD : HD + 1], 1.0
                )

            # warm-up matmuls on a zeroed tile during the input-DMA lead-in:
            # ~10us of PE activity flips the HAM clock gate to 8/8 before the
            # V projection starts, so nothing runs at the 1.2 GHz cold clock
            warm = big.tile([P, 512], BF, tag="warm", name="warm")
            nc.vector.memset(warm[:], 0.0)
            wps = mmp.tile([P, 1024], F32, tag="mm", name="wps")
            for wi in range(22):
                nc.tensor.matmul(
                    wps[:, 0:512], warm[:, 0:P], warm[:], start=True, stop=True
                )
            nc.scalar.copy(warm[0:1, 0:1], wps[0:1, 0:1])

            # ---------------- projection emitters (one [128,1024] slot each)
            def v_proj(t):
                cs = slice(t * P, (t + 1) * P)
                pv = mmp.tile([P, 1024], F32, tag="mm", name="pv")
                for m in range(2):
                    sl = slice(m * 512, (m + 1) * 512)
                    for j in range(NT):
                        nc.tensor.matmul(
                            pv[:, sl], xt[j][:, cs], wvt[j][:, sl],
                            start=(j == 0), stop=(j == NT - 1),
                        )
                dst = vs[t].rearrange("p (h c) -> p h c", c=HC)[:, :, 0:HD]
                srcv = pv.rearrange("p (h c) -> p h c", c=HD)
                # ACT is idle during the projection prologue; keep DVE free
                nc.scalar.copy(dst, srcv)

            def qk_mms(t, which):
                """Yield the 16 projection matmuls for Q or K of tile t as
                thunk-batches of 4 (filler units for the attention loop)."""
                cs = slice(t * P, (t + 1) * P)
                w = wqt if which == "q" else wkt
                pq = mmp.tile([P, 1024], F32, tag="mm", name=f"p{which}{t}")
                for m in range(2):
                    sl = slice(m * 512, (m + 1) * 512)
                    for j in range(NT):
                        nc.tensor.matmul(
                            pq[:, sl], w[j][:, cs], xt[j][:, sl],
                            start=(j == 0), stop=(j == NT - 1),
                        )
                return pq

            def rope_muls(t, which, pq):
                """RoPE part 1: evacuate PSUM with the cos/sin multiplies and
                start the 32-row-block swap DMA. Returns state for rope_add."""
                tcos = work.tile([P, S], BF, tag="tcos", name="tcos")
                tsin = work.tile([P, S], BF, tag="tsin", name="tsin")
                trot = work.tile([P, S], BF, tag="trot", name="trot")
                nc.vector.tensor_tensor(tcos[:], pq[:], cosf[:], op=MUL)
                nc.vector.tensor_tensor(tsin[:], pq[:], sinf2[:], op=MUL)
                for b4 in range(4):
                    sblk = (b4 ^ 1) * 32
                    dblk = b4 * 32
                    nc.sync.dma_start(
                        trot[dblk : dblk + 32, :], tsin[sblk : sblk + 32, :]
                    )
                return (t, which, tcos, trot)

            def rope_add(st):
                """RoPE part 2 (emitted later so the DVE doesn't head-of-line
                stall on the swap DMA). GPSIMD must stay single-program
                (PartitionBroadcast only) -- a second gp op type causes
                LIBRARY_RELOAD thrash + misexec, so this add stays on DVE."""
                t, which, tcos, trot = st
                buf_t = qt[t] if which == "q" else kt[t]
                nc.vector.tensor_tensor(buf_t[:], tcos[:], trot[:], op=ADD)

            def rope(t, which, pq):
                rope_add(rope_muls(t, which, pq))

            # ---------------- attention for one tile pair (2 heads) ---------
            # Emission pipeline per step s:
            #   ST_A(s), ST_B(s)                      (PE, row-group packed)
            #   exp+mask A(s), B(s)                   (ACT, DVE)
            #   PV_A(s-1) jb0, PV_B(s-1) jb0          (PE)
            #   PV_A(s-2) jb1, PV_B(s-2) jb1          (PE, trails for norm)
            #   filler()                              (projection MMs)
            scale = 1.0 / 8.0

            # k-tile super-steps: widths pack to exactly <=1024 score columns
            # per step, so each (head, step) is ONE [128,1024] PSUM tile and
            # ONE exp -- 5 ACT ops per head instead of 8.
            STEPS = [[0], [1, 7], [2, 6], [3, 5], [4]]
            NS = len(STEPS)

            def attention(t, filler):
                heads = []
                for hh in (0, 1):
                    heads.append({
                        "base": hh * HD,
                        "h": 2 * t + hh,
                        "otp": {
                            0: otp_pool.tile([HC, 512], F32, tag="ot", name="otp0"),
                            1: otp_pool.tile([HC, 512], F32, tag="ot", name="otp1"),
                        },
                        "pt": {},   # i -> (tile, col_offset_of_i_in_tile)
                    })

                def st_exp(hd, s):
                    base = hd["base"]
                    widths = [S - i * P for i in STEPS[s]]
                    tot = sum(widths)
                    stp = mmp.tile([P, 1024], F32, tag="mm", name="stp")
                    pt = ptp.tile([P, tot], BF, tag=f"pt{s}", name=f"pt{s}")
                    col = 0
                    for i, w in zip(STEPS[s], widths):
                        off = i * P
                        o = 0
                        while o < w:
                            wd = min(w - o, 512 - (col + o) % 512)
                            if (col + o) % 512 == 0:
                                wd = min(w - o, 512)
                            nc.tensor.matmul(
                                stp[:, col + o : col + o + wd],
                                kt[t][base : base + HD, off : off + P],
                                qt[t][base : base + HD, off + o : off + o + wd],
                                start=True, stop=True,
                            )
                            o += wd
                        hd["pt"][i] = (pt, col)
                        col += w
                    nc.scalar.activation(pt[:, :tot], stp[:, :tot], EXP, scale=scale)
                    col = 0
                    for i, w in zip(STEPS[s], widths):
                        nc.vector.tensor_tensor(
                            pt[:, col : col + P], pt[:, col : col + P],
                            maskt[:], op=MUL,
                        )
                        col += w

                def pv(hd, i, jb):
                    lo = jb * 512
                    if i > 4 * jb + 3:
                        return
                    off = i * P
                    o = max(lo, off)
                    wd = lo + 512 - o
                    pt, col = hd["pt"][i]
                    nc.tensor.matmul(
                        hd["otp"][jb][:, o - lo : o - lo + wd],
                        vs[i][:, hd["h"] * HC : (hd["h"] + 1) * HC],
                        pt[:, col + o - off : col + o - off + wd],
                        start=(i == 0),
                        stop=(i == (3 if jb == 0 else 4)),
                    )

                def norm_single(hd, jb):
                    sl = slice(jb * 512, (jb + 1) * 512)
                    den = work.tile([1, 512], F32, tag="den", name="den")
                    nc.scalar.copy(den[:], hd["otp"][jb][HD : HD + 1, :])
                    rec = work.tile([1, 512], F32, tag="rec", name="rec")
                    nc.vector.reciprocal_approx_fast(rec[:], den[:])
                    bc = work.tile([HD, 512], F32, tag="bc", name="bc")
                    nc.gpsimd.partition_broadcast(bc[:], rec[:])
                    nc.vector.tensor_tensor(
                        ot[t][hd["base"] : hd["base"] + HD, sl],
                        hd["otp"][jb][0:HD, :], bc[:],
                        op=MUL,
                    )

                # PV trails the exp by 2 steps (jb0) / 3 steps (jb1) so
                # the PE never embeds exp-latency waits in the PV matmuls
                for s in range(NS + 3):
                    if s < NS:
                        for hd in heads:
                            st_exp(hd, s)
                    for hd in heads:
                        if 0 <= s - 2 < NS:
                            for i in STEPS[s - 2]:
                                pv(hd, i, 0)
                    for hd in heads:
                        if 0 <= s - 3 < NS:
                            for i in STEPS[s - 3]:
                                pv(hd, i, 1)
                    if s == 5:
                        for hd in heads:
                            norm_single(hd, 0)
                    filler(s)
                for hd in heads:
                    norm_single(hd, 1)

            # ---------------- emission schedule -----------------------------
            # All V tiles + Q/K of tiles 0..1 (with rope) up front so
            # attention(0) starts with everything ready. Q/K 2..7 become PE
            # filler inside the attention loops: QK(t+2) fires during att(t),
            # two tiles ahead of its consumer. Each filler unit first emits
            # the PREVIOUS unit's rope-add (so the DVE never head-of-line
            # stalls on the swap DMA), then the next 16 projection matmuls +
            # rope multiplies.
            # dovetail the QK0/QK1q groups into the V phase right where the
            # wq/wk DMAs land (~22us): V0..V2 cover the xt+wv arrival window,
            # then the QK groups run while V3..V7 follow -- attention(0)
            # starts ~10us earlier than a strictly serial prologue
            for t in range(3):
                v_proj(t)
            rope(0, "q", qk_mms(0, "q"))
            rope(0, "k", qk_mms(0, "k"))
            rope(1, "q", qk_mms(1, "q"))
            for t in range(3, NT):
                v_proj(t)

            pending_add = [None]

            def flush_add():
                if pending_add[0] is not None:
                    rope_add(pending_add[0])
                    pending_add[0] = None

            def make_unit(t2, which):
                def u():
                    flush_add()
                    pq = qk_mms(t2, which)
                    pending_add[0] = rope_muls(t2, which, pq)
                return u

            filler_units = [make_unit(1, "k")] + [
                make_unit(t2, w) for t2 in range(2, NT) for w in ("q", "k")
            ]
            fill_iter = iter(filler_units)

            def filler(s):
                if s in (1, 3):
                    u = next(fill_iter, None)
                    if u is not None:
                        u()
                    else:
                        flush_add()
                elif s == 5:
                    flush_add()

            for t in range(NT):
                attention(t, filler)

            # ---- output projection: final[s, :] = sum_i ot[i][:, s]^T wo[i]
            for st in range(NT):
                cs = slice(st * P, (st + 1) * P)
                fp = mmp.tile([P, 1024], F32, tag="mm", name="fp")
                for m in range(2):
                    sl = slice(m * 512, (m + 1) * 512)
                    for i in range(NT):
                        nc.tensor.matmul(
                            fp[:, sl], ot[i][:, cs], wot[i][:, sl],
                            start=(i == 0), stop=(i == NT - 1),
                        )
                osb = work.tile([P, 1024], BF, tag="osb", name="osb")
                # tail phase: ACT is idle, keep DVE free
                nc.scalar.copy(osb[:], fp[:])
                nc.sync.dma_start(out_d[cs, :], osb[:])

    nc.compile()
    _NC_CACHE["nc"] = nc
    return nc


def _host_prep(x, wq, wk, wv, wo, freqs_cos, freqs_sin):
    import ml_dtypes

    bf16 = ml_dtypes.bfloat16

    # de-interleave RoPE pairs: permuted col c of head h maps to original
    # column h*64 + (2r if r<32 else 2(r-32)+1)
    r = np.arange(HD)
    src_local = np.where(r < 32, 2 * r, 2 * (r - 32) + 1)
    perm = (np.arange(H)[:, None] * HD + src_local[None, :]).reshape(-1)

    wq_p = np.ascontiguousarray(wq[:, perm]).astype(bf16)
    wk_p = np.ascontiguousarray(wk[:, perm]).astype(bf16)
    wv_c = np.ascontiguousarray(wv).astype(bf16)
    wo_c = np.ascontiguousarray(wo).astype(bf16)

    cos_t = np.ascontiguousarray(freqs_cos.T).astype(np.float32)  # [32, S]
    sin_t = np.ascontiguousarray(freqs_sin.T).astype(np.float32)
    cosf = np.concatenate([cos_t, cos_t, cos_t, cos_t], 0).astype(bf16)  # [128,S]
    # sinf2 = 32-row-block swap of the sign-folded sin table
    # (sinf = [-s, s, -s, s]  ->  sinf2 = [s, -s, s, -s])
    sinf2 = np.concatenate([sin_t, -sin_t, sin_t, -sin_t], 0).astype(bf16)

    kq = np.arange(P)
    mask = ((kq[:, None] // BS) <= (kq[None, :] // BS)).astype(bf16)  # [128,128]

    in_maps = []
    for b in range(NCORES):
        xt = np.ascontiguousarray(x[b].T).astype(bf16)  # [D, S]
        in_maps.append(
            {
                "xt": xt,
                "wq": wq_p,
                "wk": wk_p,
                "wv": wv_c,
                "wo": wo_c,
                "cosf": cosf,
                "sinf2": sinf2,
                "mask": mask,
            }
        )
    return in_maps


def kernel(x, wq, wk, wv, wo, freqs_cos, freqs_sin):
    global LAST_RESULT
    x = np.asarray(x, dtype=np.float32)
    wq = np.asarray(wq, dtype=np.float32)
    wk = np.asarray(wk, dtype=np.float32)
    wv = np.asarray(wv, dtype=np.float32)
    wo = np.asarray(wo, dtype=np.float32)
    freqs_cos = np.asarray(freqs_cos, dtype=np.float32)
    freqs_sin = np.asarray(freqs_sin, dtype=np.float32)

    trace = bool(os.environ.get("BASS_TRACE"))
    if trace:
        _install_axon_hooks()
        import concourse.bass_utils as bass_utils

        bass_utils.upload_artifacts = lambda tmpdir: tmpdir  # no-egress sandbox

    from concourse.bass_utils import run_bass_kernel_spmd

    nc = _build_nc()
    in_maps = _host_prep(x, wq, wk, wv, wo, freqs_cos, freqs_sin)
    res = run_bass_kernel_spmd(
        nc, in_maps, core_ids=list(range(NCORES)), trace=trace
    )
    LAST_RESULT = res
    out = np.stack([res.results[b]["out"] for b in range(NCORES)], 0)
    return out.astype(np.float32)


# revision 35
# speedup vs baseline: 1.0488x; 1.0097x over previous
"""Self-contained Trainium2 Bass kernel for batched multi-head attention
with interleaved RoPE and a block-causal mask (block size 8).

Shapes (hardcoded): x [8, 1024, 1024] f32, weights [1024, 1024] f32,
freqs_cos/sin [1024, 32] f32 -> out [8, 1024, 1024] f32.

Sharding: data-parallel over batch, one batch element per NeuronCore (8 cores).

Device algorithm (per core, matmuls in bf16):
  - host pre-transposes x -> XT [D, S] and de-interleaves the RoPE pairing by
    permuting wq/wk columns so each head's 64 dims are [32 real | 32 imag].
  - QT = Wq^T XT, KT = Wk^T XT  ([D, S] layouts, head-major rows)
  - RoPE fused with the PSUM evacuation: tc = pq*cosf and ts = pq*sinf2
    (sinf2 is the 32-row-block-swapped sin table with signs folded), the
    32-row block swap is an SBUF->SBUF DMA on ts, and qt = tc + swap(ts) on
    the DVE (emitted one filler-unit later so the DVE never head-of-line
    stalls on the swap DMA).
  - V = XT^T Wv in natural [S, D] layout with a ones-column per head
    (V' [S, 65]) so the PV matmul also produces the softmax denominator.
  - scores transposed ST[k, q] per head, k-tiles packed into 5 super-steps
    {0},{1,7},{2,6},{3,5},{4} whose staircase widths sum to exactly <=1024,
    so each (head, step) is ONE [128,1024] PSUM tile and ONE exp -- 5 ACT
    ops per head instead of 8. Both heads' score matmuls are interleaved
    piece-by-piece (row groups 0-1 / 2-3).
  - exp on ACT with the 1/8 scale folded in; block-diagonal mask applied
    multiplicatively on the DVE per k-tile segment.
  - outT[h] = V'^T @ PT accumulated per 512-query bank in PSUM ([65, 512]);
    PV matmuls trail the exp by 2 steps (jb0) / 3 steps (jb1) so the PE
    never embeds exp-latency waits.
  - normalization per head-bank: ACT copy of the PSUM ones-row (ACT is
    exp-idle in steps 5..7), DVE reciprocal_approx_fast, GPSIMD
    partition-broadcast (the only gp op type -- a second one causes
    LIBRARY_RELOAD thrash), one DVE multiply PSUM->SBUF.
  - final = outT^T @ Wo in bf16, upcast to f32 on the host.

PE density: ~10us of warm-up matmuls on a zeroed tile run during the input
DMA lead-in so the HAM clock gate is at 8/8 before the V projection starts.
The attention loops are emission-interleaved with the remaining Q/K
projection groups (QK(t+1..) fire inside att(t) at steps 1/3) so the tensor
engine always has ready work while ACT runs the exp chain. PSUM budget: one
shared [128,1024] ring (bufs=2, 4 banks) for all projections + attention
scores, and 4 single-bank [65,512] slots for the two in-flight heads' PV
accumulators.
"""

import os
import sys
import types

import numpy as np

B, S, D, H, HD, BS = 8, 1024, 1024, 16, 64, 8
P = 128
NT = D // P  # 8 partition tiles
NCORES = 8

LAST_RESULT = None  # BassKernelResults of the most recent run (for test harness)


def _install_axon_hooks():
    """Provide antenv.axon_hooks (NTFF profiling hook) when the image lacks it."""
    if "antenv.axon_hooks" in sys.modules:
        return
    try:
        import antenv
        from trn_agent_boot.trn_boot import _ntff_profile_via_ctypes

        mod = types.ModuleType("antenv.axon_hooks")
        hook = _ntff_profile_via_ctypes("/opt/axon/libaxon_pjrt.so")
        mod.get_axon_ntff_profile_hook = lambda: hook
        mod.set_axon_ntff_profile_hook = lambda h: None
        sys.modules["antenv.axon_hooks"] = mod
        antenv.axon_hooks = mod
    except Exception:
        mod = types.ModuleType("antenv.axon_hooks")
        mod.get_axon_ntff_profile_hook = lambda: None
        mod.set_axon_ntff_profile_hook = lambda h: None
        sys.modules["antenv.axon_hooks"] = mod


_NC_CACHE = {}


def _build_nc():
    """Build and compile the Bass graph (one SPMD program for all 8 cores)."""
    if "nc" in _NC_CACHE:
        return _NC_CACHE["nc"]

    import concourse.mybir as mybir
    import concourse.tile as tile
    from concourse import bacc

    BF = mybir.dt.bfloat16
    F32 = mybir.dt.float32
    MUL = mybir.AluOpType.mult
    ADD = mybir.AluOpType.add
    EXP = mybir.ActivationFunctionType.Exp

    nc = bacc.Bacc("TRN2", target_bir_lowering=False, debug=False)

    xt_d = nc.dram_tensor("xt", [D, S], BF, kind="ExternalInput")
    wq_d = nc.dram_tensor("wq", [D, D], BF, kind="ExternalInput")
    wk_d = nc.dram_tensor("wk", [D, D], BF, kind="ExternalInput")
    wv_d = nc.dram_tensor("wv", [D, D], BF, kind="ExternalInput")
    wo_d = nc.dram_tensor("wo", [D, D], BF, kind="ExternalInput")
    cos_d = nc.dram_tensor("cosf", [P, S], BF, kind="ExternalInput")
    sin2_d = nc.dram_tensor("sinf2", [P, S], BF, kind="ExternalInput")
    mask_d = nc.dram_tensor("mask", [P, P], BF, kind="ExternalInput")
    out_d = nc.dram_tensor("out", [S, D], BF, kind="ExternalOutput")

    HC = HD + 1  # 65: V columns per head incl. the ones column

    with tile.TileContext(nc) as tc:
        with (
            tc.tile_pool(name="big", bufs=1) as big,
            tc.tile_pool(name="mmp", bufs=2, space="PSUM") as mmp,
            tc.tile_pool(name="otp_pool", bufs=4, space="PSUM") as otp_pool,
            tc.tile_pool(name="work", bufs=2) as work,
            tc.tile_pool(name="ptp", bufs=3) as ptp,
        ):
            xt = [big.tile([P, S], BF, tag=f"xt{j}", name=f"xt{j}") for j in range(NT)]
            wqt = [big.tile([P, D], BF, tag=f"wq{j}", name=f"wq{j}") for j in range(NT)]
            wkt = [big.tile([P, D], BF, tag=f"wk{j}", name=f"wk{j}") for j in range(NT)]
            wvt = [big.tile([P, D], BF, tag=f"wv{j}", name=f"wv{j}") for j in range(NT)]
            wot = [big.tile([P, D], BF, tag=f"wo{j}", name=f"wo{j}") for j in range(NT)]
            qt = [big.tile([P, S], BF, tag=f"qt{t}", name=f"qt{t}") for t in range(NT)]
            kt = [big.tile([P, S], BF, tag=f"kt{t}", name=f"kt{t}") for t in range(NT)]
            vs = [big.tile([P, H * HC], BF, tag=f"vs{t}", name=f"vs{t}") for t in range(NT)]
            ot = [big.tile([P, S], BF, tag=f"ot{t}", name=f"ot{t}") for t in range(NT)]
            cosf = big.tile([P, S], BF, tag="cosf", name="cosf")
            sinf2 = big.tile([P, S], BF, tag="sinf2", name="sinf2")
            maskt = big.tile([P, P], BF, tag="mask", name="mask")

            # load order matters for the compute ramp: xt+wv feed the V
            # projection (needed before any attention), wq/wk next, wo last
            for j in range(NT):
                rs = slice(j * P, (j + 1) * P)
                nc.sync.dma_start(xt[j][:], xt_d[rs, :])
                nc.sync.dma_start(wvt[j][:], wv_d[rs, :])
            for j in range(NT):
                rs = slice(j * P, (j + 1) * P)
                nc.sync.dma_start(wqt[j][:], wq_d[rs, :])
                nc.sync.dma_start(wkt[j][:], wk_d[rs, :])
            nc.sync.dma_start(cosf[:], cos_d[:])
            nc.sync.dma_start(sinf2[:], sin2_d[:])
            nc.sync.dma_start(maskt[:], mask_d[:])
            for j in range(NT):
                rs = slice(j * P, (j + 1) * P)
                nc.sync.dma_start(wot[j][:], wo_d[rs, :])

            for t in range(NT):
                nc.vector.memset(
                    vs[t].rearrange("p (h c) -> p h c", c=HC)[:, :, HD : HD + 1], 1.0
                )

            # warm-up matmuls on a zeroed tile during the input-DMA lead-in:
            # ~10us of PE activity flips the HAM clock gate to 8/8 before the
            # V projection starts, so nothing runs at the 1.2 GHz cold clock
            warm = big.tile([P, 512], BF, tag="warm", name="warm")
            nc.vector.memset(warm[:], 0.0)
            wps = mmp.tile([P, 1024], F32, tag="mm", name="wps")
            for wi in range(22):
                nc.tensor.matmul(
                    wps[:, 0:512], warm[:, 0:P], warm[:], start=True, stop=True
                )
            nc.scalar.copy(warm[0:1, 0:1], wps[0:1, 0:1])

            # ---------------- projection emitters (one [128,1024] slot each)
            def v_proj(t):
                cs = slice(t * P, (t + 1) * P)
                pv = mmp.tile([P, 1024], F32, tag="mm", name="pv")
                for m in range(2):
                    sl = slice(m * 512, (m + 1) * 512)
                    for j in range(NT):
                        nc.tensor.matmul(
                            pv[:, sl], xt[j][:, cs], wvt[j][:, sl],
                            start=(j == 0), stop=(j == NT - 1),
                        )
                dst = vs[t].rearrange("p (h c) -> p h c", c=HC)[:, :, 0:HD]
                srcv = pv.rearrange("p (h c) -> p h c", c=HD)
                # ACT is idle during the projection prologue; keep DVE free
                nc.scalar.copy(dst, srcv)

            def qk_mms(t, which):
                """Yield the 16 projection matmuls for Q or K of tile t as
                thunk-batches of 4 (filler units for the attention loop)."""
                cs = slice(t * P, (t + 1) * P)
                w = wqt if which == "q" else wkt
                pq = mmp.tile([P, 1024], F32, tag="mm", name=f"p{which}{t}")
                for m in range(2):
                    sl = slice(m * 512, (m + 1) * 512)
                    for j in range(NT):
                        nc.tensor.matmul(
                            pq[:, sl], w[j][:, cs], xt[j][:, sl],
                            start=(j == 0), stop=(j == NT - 1),
                        )
                return pq

            def rope_muls(t, which, pq):
                """RoPE part 1: evacuate PSUM with the cos/sin multiplies and
                start the 32-row-block swap DMA. Returns state for rope_add."""
                tcos = work.tile([P, S], BF, tag="tcos", name="tcos")
                tsin = work.tile([P, S], BF, tag="tsin", name="tsin")
                trot = work.tile([P, S], BF, tag="trot", name="trot")
                nc.vector.tensor_tensor(tcos[:], pq[:], cosf[:], op=MUL)
                nc.vector.tensor_tensor(tsin[:], pq[:], sinf2[:], op=MUL)
                for b4 in range(4):
                    sblk = (b4 ^ 1) * 32
                    dblk = b4 * 32
                    nc.sync.dma_start(
                        trot[dblk : dblk + 32, :], tsin[sblk : sblk + 32, :]
                    )
                return (t, which, tcos, trot)

            def rope_add(st):
                """RoPE part 2 (emitted later so the DVE doesn't head-of-line
                stall on the swap DMA). GPSIMD must stay single-program
                (PartitionBroadcast only) -- a second gp op type causes
                LIBRARY_RELOAD thrash + misexec, so this add stays on DVE."""
                t, which, tcos, trot = st
                buf_t = qt[t] if which == "q" else kt[t]
                nc.vector.tensor_tensor(buf_t[:], tcos[:], trot[:], op=ADD)

            def rope(t, which, pq):
                rope_add(rope_muls(t, which, pq))

            # ---------------- attention for one tile pair (2 heads) ---------
            # Emission pipeline per step s:
            #   ST_A(s), ST_B(s)                      (PE, row-group packed)
            #   exp+mask A(s), B(s)                   (ACT, DVE)
            #   PV_A(s-1) jb0, PV_B(s-1) jb0          (PE)
            #   PV_A(s-2) jb1, PV_B(s-2) jb1          (PE, trails for norm)
            #   filler()                              (projection MMs)
            scale = 1.0 / 8.0

            # k-tile super-steps: widths pack to exactly <=1024 score columns
            # per step, so each (head, step) is ONE [128,1024] PSUM tile and
            # ONE exp -- 5 ACT ops per head instead of 8.
            STEPS = [[0], [1, 7], [2, 6], [3, 5], [4]]
            NS = len(STEPS)

            def attention(t, filler):
                heads = []
                for hh in (0, 1):
                    heads.append({
                        "base": hh * HD,
                        "h": 2 * t + hh,
                        "otp": {
                            0: otp_pool.tile([HC, 512], F32, tag="ot", name="otp0"),
                            1: otp_pool.tile([HC, 512], F32, tag="ot", name="otp1"),
                        },
                        "pt": {},   # i -> (tile, col_offset_of_i_in_tile)
                    })

                def st_exp(hd, s):
                    base = hd["base"]
                    widths = [S - i * P for i in STEPS[s]]
                    tot = sum(widths)
                    stp = mmp.tile([P, 1024], F32, tag="mm", name="stp")
                    pt = ptp.tile([P, tot], BF, tag=f"pt{s}", name=f"pt{s}")
                    col = 0
                    for i, w in zip(STEPS[s], widths):
                        off = i * P
                        o = 0
                        while o < w:
                            wd = min(w - o, 512 - (col + o) % 512)
                            if (col + o) % 512 == 0:
                                wd = min(w - o, 512)
                            nc.tensor.matmul(
                                stp[:, col + o : col + o + wd],
                                kt[t][base : base + HD, off : off + P],
                                qt[t][base : base + HD, off + o : off + o + wd],
                                start=True, stop=True,
                            )
                            o += wd
                        hd["pt"][i] = (pt, col)
                        col += w
                    nc.scalar.activation(pt[:, :tot], stp[:, :tot], EXP, scale=scale)
                    col = 0
                    for i, w in zip(STEPS[s], widths):
                        nc.vector.tensor_tensor(
                            pt[:, col : col + P], pt[:, col : col + P],
                            maskt[:], op=MUL,
                        )
                        col += w

                def pv(hd, i, jb):
                    lo = jb * 512
                    if i > 4 * jb + 3:
                        return
                    off = i * P
                    o = max(lo, off)
                    wd = lo + 512 - o
                    pt, col = hd["pt"][i]
                    nc.tensor.matmul(
                        hd["otp"][jb][:, o - lo : o - lo + wd],
                        vs[i][:, hd["h"] * HC : (hd["h"] + 1) * HC],
                        pt[:, col + o - off : col + o - off + wd],
                        start=(i == 0),
                        stop=(i == (3 if jb == 0 else 4)),
                    )

                def norm_single(hd, jb):
                    sl = slice(jb * 512, (jb + 1) * 512)
                    den = work.tile([1, 512], F32, tag="den", name="den")
                    if jb == 0:
                        # steps 5..7 are exp-idle on ACT
                        nc.scalar.copy(den[:], hd["otp"][jb][HD : HD + 1, :])
                    else:
                        # tile-end: keep ACT clear so the NEXT tile's first
                        # exp (which gates its score-ring release) starts
                        # immediately; DVE has headroom here
                        nc.vector.tensor_copy(den[:], hd["otp"][jb][HD : HD + 1, :])
                    rec = work.tile([1, 512], F32, tag="rec", name="rec")
                    nc.vector.reciprocal_approx_fast(rec[:], den[:])
                    bc = work.tile([HD, 512], F32, tag="bc", name="bc")
                    nc.gpsimd.partition_broadcast(bc[:], rec[:])
                    nc.vector.tensor_tensor(
                        ot[t][hd["base"] : hd["base"] + HD, sl],
                        hd["otp"][jb][0:HD, :], bc[:],
                        op=MUL,
                    )

                # PV trails the exp by 2 steps (jb0) / 3 steps (jb1) so
                # the PE never embeds exp-latency waits in the PV matmuls
                for s in range(NS + 3):
                    if s < NS:
                        for hd in heads:
                            st_exp(hd, s)
                    for hd in heads:
                        if 0 <= s - 2 < NS:
                            for i in STEPS[s - 2]:
                                pv(hd, i, 0)
                    for hd in heads:
                        if 0 <= s - 3 < NS:
                            for i in STEPS[s - 3]:
                                pv(hd, i, 1)
                    if s == 5:
                        for hd in heads:
                            norm_single(hd, 0)
                    filler(s)
                for hd in heads:
                    norm_single(hd, 1)

            # ---------------- emission schedule -----------------------------
            # All V tiles + Q/K of tiles 0..1 (with rope) up front so
            # attention(0) starts with everything ready. Q/K 2..7 become PE
            # filler inside the attention loops: QK(t+2) fires during att(t),
            # two tiles ahead of its consumer. Each filler unit first emits
            # the PREVIOUS unit's rope-add (so the DVE never head-of-line
            # stalls on the swap DMA), then the next 16 projection matmuls +
            # rope multiplies.
            # dovetail the QK0/QK1q groups into the V phase right where the
            # wq/wk DMAs land (~22us): V0..V2 cover the xt+wv arrival window,
            # then the QK groups run while V3..V7 follow -- attention(0)
            # starts ~10us earlier than a strictly serial prologue
            for t in range(3):
                v_proj(t)
            rope(0, "q", qk_mms(0, "q"))
            rope(0, "k", qk_mms(0, "k"))
            rope(1, "q", qk_mms(1, "q"))
            for t in range(3, NT):
                v_proj(t)

            pending_add = [None]

            def flush_add():
                if pending_add[0] is not None:
                    rope_add(pending_add[0])
                    pending_add[0] = None

            def make_unit(t2, which):
                def u():
                    flush_add()
                    pq = qk_mms(t2, which)
                    pending_add[0] = rope_muls(t2, which, pq)
                return u

            filler_units = [make_unit(1, "k")] + [
                make_unit(t2, w) for t2 in range(2, NT) for w in ("q", "k")
            ]
            fill_iter = iter(filler_units)

            def filler(s):
                if s in (1, 3):
                    u = next(fill_iter, None)
                    if u is not None:
                        u()
                    else:
                        flush_add()
                elif s == 5:
                    flush_add()

            for t in range(NT):
                attention(t, filler)

            # ---- output projection: final[s, :] = sum_i ot[i][:, s]^T wo[i]
            for st in range(NT):
                cs = slice(st * P, (st + 1) * P)
                fp = mmp.tile([P, 1024], F32, tag="mm", name="fp")
                for m in range(2):
                    sl = slice(m * 512, (m + 1) * 512)
                    for i in range(NT):
                        nc.tensor.matmul(
                            fp[:, sl], ot[i][:, cs], wot[i][:, sl],
                            start=(i == 0), stop=(i == NT - 1),
                        )
                osb = work.tile([P, 1024], BF, tag="osb", name="osb")
                # tail phase: ACT is idle, keep DVE free
                nc.scalar.copy(osb[:], fp[:])
                nc.sync.dma_start(out_d[cs, :], osb[:])

    nc.compile()
    _NC_CACHE["nc"] = nc
    return nc


def _host_prep(x, wq, wk, wv, wo, freqs_cos, freqs_sin):
    import ml_dtypes

    bf16 = ml_dtypes.bfloat16

    # de-interleave RoPE pairs: permuted col c of head h maps to original
    # column h*64 + (2r if r<32 else 2(r-32)+1)
    r = np.arange(HD)
    src_local = np.where(r < 32, 2 * r, 2 * (r - 32) + 1)
    perm = (np.arange(H)[:, None] * HD + src_local[None, :]).reshape(-1)

    wq_p = np.ascontiguousarray(wq[:, perm]).astype(bf16)
    wk_p = np.ascontiguousarray(wk[:, perm]).astype(bf16)
    wv_c = np.ascontiguousarray(wv).astype(bf16)
    wo_c = np.ascontiguousarray(wo).astype(bf16)

    cos_t = np.ascontiguousarray(freqs_cos.T).astype(np.float32)  # [32, S]
    sin_t = np.ascontiguousarray(freqs_sin.T).astype(np.float32)
    cosf = np.concatenate([cos_t, cos_t, cos_t, cos_t], 0).astype(bf16)  # [128,S]
    # sinf2 = 32-row-block swap of the sign-folded sin table
    # (sinf = [-s, s, -s, s]  ->  sinf2 = [s, -s, s, -s])
    sinf2 = np.concatenate([sin_t, -sin_t, sin_t, -sin_t], 0).astype(bf16)

    kq = np.arange(P)
    mask = ((kq[:, None] // BS) <= (kq[None, :] // BS)).astype(bf16)  # [128,128]

    in_maps = []
    for b in range(NCORES):
        xt = np.ascontiguousarray(x[b].T).astype(bf16)  # [D, S]
        in_maps.append(
            {
                "xt": xt,
                "wq": wq_p,
                "wk": wk_p,
                "wv": wv_c,
                "wo": wo_c,
                "cosf": cosf,
                "sinf2": sinf2,
                "mask": mask,
            }
        )
    return in_maps


def kernel(x, wq, wk, wv, wo, freqs_cos, freqs_sin):
    global LAST_RESULT
    x = np.asarray(x, dtype=np.float32)
    wq = np.asarray(wq, dtype=np.float32)
    wk = np.asarray(wk, dtype=np.float32)
    wv = np.asarray(wv, dtype=np.float32)
    wo = np.asarray(wo, dtype=np.float32)
    freqs_cos = np.asarray(freqs_cos, dtype=np.float32)
    freqs_sin = np.asarray(freqs_sin, dtype=np.float32)

    trace = bool(os.environ.get("BASS_TRACE"))
    if trace:
        _install_axon_hooks()
        import concourse.bass_utils as bass_utils

        bass_utils.upload_artifacts = lambda tmpdir: tmpdir  # no-egress sandbox

    from concourse.bass_utils import run_bass_kernel_spmd

    nc = _build_nc()
    in_maps = _host_prep(x, wq, wk, wv, wo, freqs_cos, freqs_sin)
    res = run_bass_kernel_spmd(
        nc, in_maps, core_ids=list(range(NCORES)), trace=trace
    )
    LAST_RESULT = res
    out = np.stack([res.results[b]["out"] for b in range(NCORES)], 0)
    return out.astype(np.float32)


# revision 36
# speedup vs baseline: 1.0596x; 1.0103x over previous
"""Self-contained Trainium2 Bass kernel for batched multi-head attention
with interleaved RoPE and a block-causal mask (block size 8).

Shapes (hardcoded): x [8, 1024, 1024] f32, weights [1024, 1024] f32,
freqs_cos/sin [1024, 32] f32 -> out [8, 1024, 1024] f32.

Sharding: data-parallel over batch, one batch element per NeuronCore (8 cores).

Device algorithm (per core, matmuls in bf16):
  - host pre-transposes x -> XT [D, S] and de-interleaves the RoPE pairing by
    permuting wq/wk columns so each head's 64 dims are [32 real | 32 imag].
  - QT = Wq^T XT, KT = Wk^T XT  ([D, S] layouts, head-major rows)
  - RoPE fused with the PSUM evacuation: tc = pq*cosf and ts = pq*sinf2
    (sinf2 is the 32-row-block-swapped sin table with signs folded), the
    32-row block swap is an SBUF->SBUF DMA on ts, and qt = tc + swap(ts) on
    the DVE (emitted one filler-unit later so the DVE never head-of-line
    stalls on the swap DMA).
  - V = XT^T Wv in natural [S, D] layout with a ones-column per head
    (V' [S, 65]) so the PV matmul also produces the softmax denominator.
  - scores transposed ST[k, q] per head, k-tiles packed into 5 super-steps
    {0},{1,7},{2,6},{3,5},{4} whose staircase widths sum to exactly <=1024,
    so each (head, step) is ONE [128,1024] PSUM tile and ONE exp -- 5 ACT
    ops per head instead of 8. Both heads' score matmuls are interleaved
    piece-by-piece (row groups 0-1 / 2-3).
  - exp on ACT with the 1/8 scale folded in; block-diagonal mask applied
    multiplicatively on the DVE per k-tile segment.
  - outT[h] = V'^T @ PT accumulated per 512-query bank in PSUM ([65, 512]);
    PV matmuls trail the exp by 2 steps (jb0) / 3 steps (jb1) so the PE
    never embeds exp-latency waits.
  - normalization per head-bank: ACT copy of the PSUM ones-row (ACT is
    exp-idle in steps 5..7), DVE reciprocal_approx_fast, GPSIMD
    partition-broadcast (the only gp op type -- a second one causes
    LIBRARY_RELOAD thrash), one DVE multiply PSUM->SBUF.
  - final = outT^T @ Wo in bf16, upcast to f32 on the host.

PE density: ~10us of warm-up matmuls on a zeroed tile run during the input
DMA lead-in so the HAM clock gate is at 8/8 before the V projection starts.
The attention loops are emission-interleaved with the remaining Q/K
projection groups (QK(t+1..) fire inside att(t) at steps 1/3) so the tensor
engine always has ready work while ACT runs the exp chain. PSUM budget: one
shared [128,1024] ring (bufs=2, 4 banks) for all projections + attention
scores, and 4 single-bank [65,512] slots for the two in-flight heads' PV
accumulators.
"""

import os
import sys
import types

import numpy as np

B, S, D, H, HD, BS = 8, 1024, 1024, 16, 64, 8
P = 128
NT = D // P  # 8 partition tiles
NCORES = 8

LAST_RESULT = None  # BassKernelResults of the most recent run (for test harness)


def _install_axon_hooks():
    """Provide antenv.axon_hooks (NTFF profiling hook) when the image lacks it."""
    if "antenv.axon_hooks" in sys.modules:
        return
    try:
        import antenv
        from trn_agent_boot.trn_boot import _ntff_profile_via_ctypes

        mod = types.ModuleType("antenv.axon_hooks")
        hook = _ntff_profile_via_ctypes("/opt/axon/libaxon_pjrt.so")
        mod.get_axon_ntff_profile_hook = lambda: hook
        mod.set_axon_ntff_profile_hook = lambda h: None
        sys.modules["antenv.axon_hooks"] = mod
        antenv.axon_hooks = mod
    except Exception:
        mod = types.ModuleType("antenv.axon_hooks")
        mod.get_axon_ntff_profile_hook = lambda: None
        mod.set_axon_ntff_profile_hook = lambda h: None
        sys.modules["antenv.axon_hooks"] = mod


_NC_CACHE = {}


def _build_nc():
    """Build and compile the Bass graph (one SPMD program for all 8 cores)."""
    if "nc" in _NC_CACHE:
        return _NC_CACHE["nc"]

    import concourse.mybir as mybir
    import concourse.tile as tile
    from concourse import bacc

    BF = mybir.dt.bfloat16
    F32 = mybir.dt.float32
    MUL = mybir.AluOpType.mult
    ADD = mybir.AluOpType.add
    EXP = mybir.ActivationFunctionType.Exp

    nc = bacc.Bacc("TRN2", target_bir_lowering=False, debug=False)

    xt_d = nc.dram_tensor("xt", [D, S], BF, kind="ExternalInput")
    wq_d = nc.dram_tensor("wq", [D, D], BF, kind="ExternalInput")
    wk_d = nc.dram_tensor("wk", [D, D], BF, kind="ExternalInput")
    wv_d = nc.dram_tensor("wv", [D, D], BF, kind="ExternalInput")
    wo_d = nc.dram_tensor("wo", [D, D], BF, kind="ExternalInput")
    cos_d = nc.dram_tensor("cosf", [P, S], BF, kind="ExternalInput")
    sin2_d = nc.dram_tensor("sinf2", [P, S], BF, kind="ExternalInput")
    mask_d = nc.dram_tensor("mask", [P, P], BF, kind="ExternalInput")
    out_d = nc.dram_tensor("out", [S, D], BF, kind="ExternalOutput")

    HC = HD + 1  # 65: V columns per head incl. the ones column

    with tile.TileContext(nc) as tc:
        with (
            tc.tile_pool(name="big", bufs=1) as big,
            tc.tile_pool(name="mmp", bufs=2, space="PSUM") as mmp,
            tc.tile_pool(name="otp_pool", bufs=4, space="PSUM") as otp_pool,
            tc.tile_pool(name="work", bufs=2) as work,
            tc.tile_pool(name="ptp", bufs=3) as ptp,
        ):
            xt = [big.tile([P, S], BF, tag=f"xt{j}", name=f"xt{j}") for j in range(NT)]
            wqt = [big.tile([P, D], BF, tag=f"wq{j}", name=f"wq{j}") for j in range(NT)]
            wkt = [big.tile([P, D], BF, tag=f"wk{j}", name=f"wk{j}") for j in range(NT)]
            wvt = [big.tile([P, D], BF, tag=f"wv{j}", name=f"wv{j}") for j in range(NT)]
            wot = [big.tile([P, D], BF, tag=f"wo{j}", name=f"wo{j}") for j in range(NT)]
            qt = [big.tile([P, S], BF, tag=f"qt{t}", name=f"qt{t}") for t in range(NT)]
            kt = [big.tile([P, S], BF, tag=f"kt{t}", name=f"kt{t}") for t in range(NT)]
            vs = [big.tile([P, H * HC], BF, tag=f"vs{t}", name=f"vs{t}") for t in range(NT)]
            ot = [big.tile([P, S], BF, tag=f"ot{t}", name=f"ot{t}") for t in range(NT)]
            cosf = big.tile([P, S], BF, tag="cosf", name="cosf")
            sinf2 = big.tile([P, S], BF, tag="sinf2", name="sinf2")
            maskt = big.tile([P, P], BF, tag="mask", name="mask")

            # load order matters for the compute ramp: xt+wv feed the V
            # projection (needed before any attention), wq/wk next, wo last
            for j in range(NT):
                rs = slice(j * P, (j + 1) * P)
                nc.sync.dma_start(xt[j][:], xt_d[rs, :])
                nc.sync.dma_start(wvt[j][:], wv_d[rs, :])
            for j in range(NT):
                rs = slice(j * P, (j + 1) * P)
                nc.sync.dma_start(wqt[j][:], wq_d[rs, :])
                nc.sync.dma_start(wkt[j][:], wk_d[rs, :])
            nc.sync.dma_start(cosf[:], cos_d[:])
            nc.sync.dma_start(sinf2[:], sin2_d[:])
            nc.sync.dma_start(maskt[:], mask_d[:])
            for j in range(NT):
                rs = slice(j * P, (j + 1) * P)
                nc.sync.dma_start(wot[j][:], wo_d[rs, :])

            for t in range(NT):
                nc.vector.memset(
                    vs[t].rearrange("p (h c) -> p h c", c=HC)[:, :, HD : HD + 1], 1.0
                )

            # warm-up matmuls on a zeroed tile during the input-DMA lead-in:
            # ~10us of PE activity flips the HAM clock gate to 8/8 before the
            # V projection starts, so nothing runs at the 1.2 GHz cold clock
            warm = big.tile([P, 512], BF, tag="warm", name="warm")
            nc.vector.memset(warm[:], 0.0)
            wps = mmp.tile([P, 1024], F32, tag="mm", name="wps")
            for wi in range(22):
                nc.tensor.matmul(
                    wps[:, 0:512], warm[:, 0:P], warm[:], start=True, stop=True
                )
            nc.scalar.copy(warm[0:1, 0:1], wps[0:1, 0:1])

            # ---------------- projection emitters (one [128,1024] slot each)
            def v_proj(t):
                cs = slice(t * P, (t + 1) * P)
                pv = mmp.tile([P, 1024], F32, tag="mm", name="pv")
                for m in range(2):
                    sl = slice(m * 512, (m + 1) * 512)
                    for j in range(NT):
                        nc.tensor.matmul(
                            pv[:, sl], xt[j][:, cs], wvt[j][:, sl],
                            start=(j == 0), stop=(j == NT - 1),
                        )
                dst = vs[t].rearrange("p (h c) -> p h c", c=HC)[:, :, 0:HD]
                srcv = pv.rearrange("p (h c) -> p h c", c=HD)
                if t < 6:
                    # ACT is idle during the projection prologue
                    nc.scalar.copy(dst, srcv)
                else:
                    # V6/V7 evacuate right before attention(0): keep ACT clear
                    # so the first exps start immediately (same rule as the
                    # jb1 den copies at tile boundaries)
                    nc.vector.tensor_copy(dst, srcv)

            def qk_mms(t, which):
                """Yield the 16 projection matmuls for Q or K of tile t as
                thunk-batches of 4 (filler units for the attention loop)."""
                cs = slice(t * P, (t + 1) * P)
                w = wqt if which == "q" else wkt
                pq = mmp.tile([P, 1024], F32, tag="mm", name=f"p{which}{t}")
                for m in range(2):
                    sl = slice(m * 512, (m + 1) * 512)
                    for j in range(NT):
                        nc.tensor.matmul(
                            pq[:, sl], w[j][:, cs], xt[j][:, sl],
                            start=(j == 0), stop=(j == NT - 1),
                        )
                return pq

            def rope_muls(t, which, pq):
                """RoPE part 1: evacuate PSUM with the cos/sin multiplies and
                start the 32-row-block swap DMA. Returns state for rope_add."""
                tcos = work.tile([P, S], BF, tag="tcos", name="tcos")
                tsin = work.tile([P, S], BF, tag="tsin", name="tsin")
                trot = work.tile([P, S], BF, tag="trot", name="trot")
                nc.vector.tensor_tensor(tcos[:], pq[:], cosf[:], op=MUL)
                nc.vector.tensor_tensor(tsin[:], pq[:], sinf2[:], op=MUL)
                for b4 in range(4):
                    sblk = (b4 ^ 1) * 32
                    dblk = b4 * 32
                    nc.sync.dma_start(
                        trot[dblk : dblk + 32, :], tsin[sblk : sblk + 32, :]
                    )
                return (t, which, tcos, trot)

            def rope_add(st):
                """RoPE part 2 (emitted later so the DVE doesn't head-of-line
                stall on the swap DMA). GPSIMD must stay single-program
                (PartitionBroadcast only) -- a second gp op type causes
                LIBRARY_RELOAD thrash + misexec, so this add stays on DVE."""
                t, which, tcos, trot = st
                buf_t = qt[t] if which == "q" else kt[t]
                nc.vector.tensor_tensor(buf_t[:], tcos[:], trot[:], op=ADD)

            def rope(t, which, pq):
                rope_add(rope_muls(t, which, pq))

            # ---------------- attention for one tile pair (2 heads) ---------
            # Emission pipeline per step s:
            #   ST_A(s), ST_B(s)                      (PE, row-group packed)
            #   exp+mask A(s), B(s)                   (ACT, DVE)
            #   PV_A(s-1) jb0, PV_B(s-1) jb0          (PE)
            #   PV_A(s-2) jb1, PV_B(s-2) jb1          (PE, trails for norm)
            #   filler()                              (projection MMs)
            scale = 1.0 / 8.0

            # k-tile super-steps: widths pack to exactly <=1024 score columns
            # per step, so each (head, step) is ONE [128,1024] PSUM tile and
            # ONE exp -- 5 ACT ops per head instead of 8.
            STEPS = [[0], [1, 7], [2, 6], [3, 5], [4]]
            NS = len(STEPS)

            def attention(t, filler):
                heads = []
                for hh in (0, 1):
                    heads.append({
                        "base": hh * HD,
                        "h": 2 * t + hh,
                        "otp": {
                            0: otp_pool.tile([HC, 512], F32, tag="ot", name="otp0"),
                            1: otp_pool.tile([HC, 512], F32, tag="ot", name="otp1"),
                        },
                        "pt": {},   # i -> (tile, col_offset_of_i_in_tile)
                    })

                def st_exp(hd, s):
                    base = hd["base"]
                    widths = [S - i * P for i in STEPS[s]]
                    tot = sum(widths)
                    stp = mmp.tile([P, 1024], F32, tag="mm", name="stp")
                    pt = ptp.tile([P, tot], BF, tag=f"pt{s}", name=f"pt{s}")
                    col = 0
                    for i, w in zip(STEPS[s], widths):
                        off = i * P
                        o = 0
                        while o < w:
                            wd = min(w - o, 512 - (col + o) % 512)
                            if (col + o) % 512 == 0:
                                wd = min(w - o, 512)
                            nc.tensor.matmul(
                                stp[:, col + o : col + o + wd],
                                kt[t][base : base + HD, off : off + P],
                                qt[t][base : base + HD, off + o : off + o + wd],
                                start=True, stop=True,
                            )
                            o += wd
                        hd["pt"][i] = (pt, col)
                        col += w
                    nc.scalar.activation(pt[:, :tot], stp[:, :tot], EXP, scale=scale)
                    col = 0
                    for i, w in zip(STEPS[s], widths):
                        nc.vector.tensor_tensor(
                            pt[:, col : col + P], pt[:, col : col + P],
                            maskt[:], op=MUL,
                        )
                        col += w

                def pv(hd, i, jb):
                    lo = jb * 512
                    if i > 4 * jb + 3:
                        return
                    off = i * P
                    o = max(lo, off)
                    wd = lo + 512 - o
                    pt, col = hd["pt"][i]
                    nc.tensor.matmul(
                        hd["otp"][jb][:, o - lo : o - lo + wd],
                        vs[i][:, hd["h"] * HC : (hd["h"] + 1) * HC],
                        pt[:, col + o - off : col + o - off + wd],
                        start=(i == 0),
                        stop=(i == (3 if jb == 0 else 4)),
                    )

                def norm_single(hd, jb):
                    sl = slice(jb * 512, (jb + 1) * 512)
                    den = work.tile([1, 512], F32, tag="den", name="den")
                    if jb == 0:
                        # steps 5..7 are exp-idle on ACT
                        nc.scalar.copy(den[:], hd["otp"][jb][HD : HD + 1, :])
                    else:
                        # tile-end: keep ACT clear so the NEXT tile's first
                        # exp (which gates its score-ring release) starts
                        # immediately; DVE has headroom here
                        nc.vector.tensor_copy(den[:], hd["otp"][jb][HD : HD + 1, :])
                    rec = work.tile([1, 512], F32, tag="rec", name="rec")
                    nc.vector.reciprocal_approx_fast(rec[:], den[:])
                    bc = work.tile([HD, 512], F32, tag="bc", name="bc")
                    nc.gpsimd.partition_broadcast(bc[:], rec[:])
                    nc.vector.tensor_tensor(
                        ot[t][hd["base"] : hd["base"] + HD, sl],
                        hd["otp"][jb][0:HD, :], bc[:],
                        op=MUL,
                    )

                # PV trails the exp by 2 steps (jb0) / 3 steps (jb1) so
                # the PE never embeds exp-latency waits in the PV matmuls
                for s in range(NS + 3):
                    if s < NS:
                        for hd in heads:
                            st_exp(hd, s)
                    for hd in heads:
                        if 0 <= s - 2 < NS:
                            for i in STEPS[s - 2]:
                                pv(hd, i, 0)
                    for hd in heads:
                        if 0 <= s - 3 < NS:
                            for i in STEPS[s - 3]:
                                pv(hd, i, 1)
                    if s == 5:
                        for hd in heads:
                            norm_single(hd, 0)
                    filler(s)
                for hd in heads:
                    norm_single(hd, 1)

            # ---------------- emission schedule -----------------------------
            # All V tiles + Q/K of tiles 0..1 (with rope) up front so
            # attention(0) starts with everything ready. Q/K 2..7 become PE
            # filler inside the attention loops: QK(t+2) fires during att(t),
            # two tiles ahead of its consumer. Each filler unit first emits
            # the PREVIOUS unit's rope-add (so the DVE never head-of-line
            # stalls on the swap DMA), then the next 16 projection matmuls +
            # rope multiplies.
            # dovetail the QK0/QK1q groups into the V phase right where the
            # wq/wk DMAs land (~22us): V0..V2 cover the xt+wv arrival window,
            # then the QK groups run while V3..V7 follow -- attention(0)
            # starts ~10us earlier than a strictly serial prologue
            for t in range(3):
                v_proj(t)
            rope(0, "q", qk_mms(0, "q"))
            rope(0, "k", qk_mms(0, "k"))
            rope(1, "q", qk_mms(1, "q"))
            for t in range(3, NT):
                v_proj(t)

            pending_add = [None]

            def flush_add():
                if pending_add[0] is not None:
                    rope_add(pending_add[0])
                    pending_add[0] = None

            def make_unit(t2, which):
                def u():
                    flush_add()
                    pq = qk_mms(t2, which)
                    pending_add[0] = rope_muls(t2, which, pq)
                return u

            filler_units = [make_unit(1, "k")] + [
                make_unit(t2, w) for t2 in range(2, NT) for w in ("q", "k")
            ]
            fill_iter = iter(filler_units)

            def filler(s):
                if s in (1, 3):
                    u = next(fill_iter, None)
                    if u is not None:
                        u()
                    else:
                        flush_add()
                elif s == 5:
                    flush_add()

            for t in range(NT):
                attention(t, filler)

            # ---- output projection: final[s, :] = sum_i ot[i][:, s]^T wo[i]
            for st in range(NT):
                cs = slice(st * P, (st + 1) * P)
                fp = mmp.tile([P, 1024], F32, tag="mm", name="fp")
                for m in range(2):
                    sl = slice(m * 512, (m + 1) * 512)
                    for i in range(NT):
                        nc.tensor.matmul(
                            fp[:, sl], ot[i][:, cs], wot[i][:, sl],
                            start=(i == 0), stop=(i == NT - 1),
                        )
                osb = work.tile([P, 1024], BF, tag="osb", name="osb")
                # tail phase: ACT is idle, keep DVE free
                nc.scalar.copy(osb[:], fp[:])
                nc.sync.dma_start(out_d[cs, :], osb[:])

    nc.compile()
    _NC_CACHE["nc"] = nc
    return nc


def _host_prep(x, wq, wk, wv, wo, freqs_cos, freqs_sin):
    import ml_dtypes

    bf16 = ml_dtypes.bfloat16

    # de-interleave RoPE pairs: permuted col c of head h maps to original
    # column h*64 + (2r if r<32 else 2(r-32)+1)
    r = np.arange(HD)
    src_local = np.where(r < 32, 2 * r, 2 * (r - 32) + 1)
    perm = (np.arange(H)[:, None] * HD + src_local[None, :]).reshape(-1)

    wq_p = np.ascontiguousarray(wq[:, perm]).astype(bf16)
    wk_p = np.ascontiguousarray(wk[:, perm]).astype(bf16)
    wv_c = np.ascontiguousarray(wv).astype(bf16)
    wo_c = np.ascontiguousarray(wo).astype(bf16)

    cos_t = np.ascontiguousarray(freqs_cos.T).astype(np.float32)  # [32, S]
    sin_t = np.ascontiguousarray(freqs_sin.T).astype(np.float32)
    cosf = np.concatenate([cos_t, cos_t, cos_t, cos_t], 0).astype(bf16)  # [128,S]
    # sinf2 = 32-row-block swap of the sign-folded sin table
    # (sinf = [-s, s, -s, s]  ->  sinf2 = [s, -s, s, -s])
    sinf2 = np.concatenate([sin_t, -sin_t, sin_t, -sin_t], 0).astype(bf16)

    kq = np.arange(P)
    mask = ((kq[:, None] // BS) <= (kq[None, :] // BS)).astype(bf16)  # [128,128]

    in_maps = []
    for b in range(NCORES):
        xt = np.ascontiguousarray(x[b].T).astype(bf16)  # [D, S]
        in_maps.append(
            {
                "xt": xt,
                "wq": wq_p,
                "wk": wk_p,
                "wv": wv_c,
                "wo": wo_c,
                "cosf": cosf,
                "sinf2": sinf2,
                "mask": mask,
            }
        )
    return in_maps


def kernel(x, wq, wk, wv, wo, freqs_cos, freqs_sin):
    global LAST_RESULT
    x = np.asarray(x, dtype=np.float32)
    wq = np.asarray(wq, dtype=np.float32)
    wk = np.asarray(wk, dtype=np.float32)
    wv = np.asarray(wv, dtype=np.float32)
    wo = np.asarray(wo, dtype=np.float32)
    freqs_cos = np.asarray(freqs_cos, dtype=np.float32)
    freqs_sin = np.asarray(freqs_sin, dtype=np.float32)

    trace = bool(os.environ.get("BASS_TRACE"))
    if trace:
        _install_axon_hooks()
        import concourse.bass_utils as bass_utils

        bass_utils.upload_artifacts = lambda tmpdir: tmpdir  # no-egress sandbox

    from concourse.bass_utils import run_bass_kernel_spmd

    nc = _build_nc()
    in_maps = _host_prep(x, wq, wk, wv, wo, freqs_cos, freqs_sin)
    res = run_bass_kernel_spmd(
        nc, in_maps, core_ids=list(range(NCORES)), trace=trace
    )
    LAST_RESULT = res
    out = np.stack([res.results[b]["out"] for b in range(NCORES)], 0)
    return out.astype(np.float32)
